# revision 9
# baseline (speedup 1.0000x reference)
"""Trainium2 Bass kernel for nn_LCNSpiking_58162447123130.

Math (verified to rel-err ~2e-3 against the oracle):

1. The "spiking" update carries zero state (syn = ALPHA*0 + cur, mem =
   BETA*0 + syn, reset = 0), so each LCN layer is a purely LINEAR map:
   h_out = h @ S_l + b_l with S_l[knn_l[j,k], j] = w_l[j,k].
2. Only the last timestep reaches the output, so
   out = x[:, -1, :] @ M + c with M = S0 S1 S2 S3 S4 Wfc ([14400, 2]),
   folded on the host in float64 from the tiny weight/index tables.

Device kernel (per core, 8-way split of the 14400 contraction axis):
stream a packed bf16 [1800, 34] tile (cols 0:32 = x_t slice, cols
32:34 = M slice) through one HWDGE DMA on the sync queue, 15
accumulating [K=120] x [32|2] bf16 matmuls on PE into a single fp32
PSUM tile, DVE copy PSUM->SBUF, sync-queue DMA of the [32, 2] partial
back to DRAM; the host sums the 8 partials (+ bias chain) in fp64.

bf16 operands halve the DMA payload and run PE at 1 cycle/row (fp32
needs the LOW/HIGH double pump); the fp32 PSUM accumulator keeps the
dot-product error ~2e-3, well inside the 2e-2 gate.

The four bass-preamble constant-MEMSETs (const-float32-0.0 etc.) are
dead code for this kernel - nothing reads the constant tiles - so they
are dropped from the entry block before compile.
"""

import numpy as np
import ml_dtypes

N_CORES = 8
B = 32                      # batch
D = 14400                   # layer-0 input dim
PER_CORE = D // N_CORES     # 1800 contraction elements per core
P = 120                     # SBUF partitions used (1800 = 120 * 15)
CHUNKS = PER_CORE // P      # 15 matmul accumulation steps
W = B + 2                   # packed row width: 32 x cols + 2 m cols
DIMS_IN = [14400, 7200, 3600, 1800, 900]

_compiled_nc = None


def _build_nc():
    import concourse.bass as bass
    import concourse.mybir as mybir

    dt = mybir.dt.bfloat16
    nc = bass.Bass()
    xm = nc.declare_dram_parameter("xm", [PER_CORE, W], dt, isOutput=False)
    out = nc.declare_dram_parameter("out", [B, 2], mybir.dt.float32, isOutput=True)

    with (
        nc.Block(no_gpsimd_drain=True) as block,
        nc.semaphore("sq_sem") as sq_sem,
        nc.semaphore("pe_sem") as pe_sem,
        nc.semaphore("cp_sem") as cp_sem,
        nc.sbuf_tensor("t", [P, CHUNKS * W], dt) as t,
        nc.sbuf_tensor("ot", [B, 2], mybir.dt.float32) as ot,
        nc.psum_tensor("ps", [B, 2], mybir.dt.float32) as ps,
        nc.psum_tensor("ps_warm", [B, 2], mybir.dt.float32) as ps_warm,
    ):
        @block.sync
        def _(sync):
            # One DMA covers both operands: partition p holds contraction
            # rows [15p, 15p+15) as 15 chunks of 34 bf16 (1020 B contiguous
            # per partition on both sides).
            sync.dma_start(
                out=t[:, :].rearrange("p (c n) -> p c n", c=CHUNKS),
                in_=xm[:, :].rearrange("(p c) n -> p c n", p=P),
            ).then_inc(sq_sem, 16)
            sync.wait_ge(cp_sem, 1)
            # Final store; the runtime's model-completion drain covers the
            # in-flight DMA, so no completion wait on the hot path.
            sync.dma_start(out=out[:, :], in_=ot[:, :]).then_inc(sq_sem, 16)

        @block.tensor
        def _(tensor):
            tensor.wait_ge(sq_sem, 16)
            mm = None
            for c in range(CHUNKS):
                # psum[32, 2] += t[:, c, 0:32].T @ t[:, c, 32:34]
                mm = nc.tensor.matmul(
                    ps[:, :],
                    t[:, c * W : c * W + B],
                    t[:, c * W + B : c * W + W],
                    start=(c == 0),
                    stop=(c == CHUNKS - 1),
                )
            mm.then_inc(pe_sem, 1)
            # Keep the PE sequencer at its warm p-state through the
            # copy/store tail: the NEFF epilogue's per-semaphore reset
            # chain on PE (51 instructions) dispatches measurably faster
            # when PE was recently busy.  Scratch matmuls into an unread
            # PSUM bank; sized to finish before the store dispatch does,
            # so PE is never the last engine into the exit barrier.
            for _ in range(30):
                nc.tensor.matmul(
                    ps_warm[:, :],
                    t[:, 0:B],
                    t[:, B:W],
                    start=True,
                    stop=True,
                )

        @block.vector
        def _(vector):
            vector.wait_ge(pe_sem, 1)
            nc.vector.tensor_copy(ot[:, :], ps[:, :]).then_inc(cp_sem, 1)

    # Drop the unused bass constant-tile memsets (dead code here).
    entry = nc.m.functions[0].blocks[0]
    entry.instructions = [
        i for i in entry.instructions if not isinstance(i, mybir.InstMemset)
    ]
    # Drop the Block-exit SP drain: it stalls ~370ns on the just-dispatched
    # store's DGE state, and both the walrus epilogue's own per-engine
    # drain and the runtime's model-completion drain already cover it.
    for blk in nc.m.functions[0].blocks:
        if blk.name.endswith("_end"):
            blk.instructions = [
                i for i in blk.instructions
                if not (
                    isinstance(i, mybir.InstDrain)
                    and i.engine == mybir.EngineType.SP
                    and i.sync_info is None
                )
            ]
    return nc


def _get_nc():
    global _compiled_nc
    if _compiled_nc is None:
        _compiled_nc = _build_nc()
    return _compiled_nc


def _fold(inputs):
    """Collapse the linear layer chain into M [14400, 2] and bias c [2]."""
    M = np.asarray(inputs["Wfc"]).astype(np.float64)
    c = np.asarray(inputs["bfc"]).astype(np.float64)
    for l in (4, 3, 2, 1, 0):
        knn = np.asarray(inputs[f"knn{l}"]).reshape(-1)
        w = np.asarray(inputs[f"w{l}"]).astype(np.float64)
        b = np.asarray(inputs[f"b{l}"]).astype(np.float64)
        c = (b @ M).ravel() + c
        Mnew = np.zeros((DIMS_IN[l], M.shape[1]), dtype=np.float64)
        np.add.at(Mnew, knn, (w[:, :, None] * M[:, None, :]).reshape(-1, M.shape[1]))
        M = Mnew
    return M.astype(np.float32), c


def kernel(**inputs) -> np.ndarray:
    from concourse.bass_utils import run_bass_kernel_spmd

    x = np.asarray(inputs["x"], dtype=np.float32)
    M, c = _fold(inputs)

    # Only the last timestep reaches the output; ship it transposed so the
    # contraction dim lands on SBUF partitions, packed next to the M slice.
    packed = np.empty((D, W), dtype=ml_dtypes.bfloat16)
    packed[:, :B] = x[:, -1, :].T.astype(ml_dtypes.bfloat16)
    packed[:, B:] = M.astype(ml_dtypes.bfloat16)

    nc = _get_nc()
    in_maps = [
        {"xm": packed[k * PER_CORE : (k + 1) * PER_CORE]}
        for k in range(N_CORES)
    ]
    res = run_bass_kernel_spmd(nc, in_maps, list(range(N_CORES))).results
    out = np.zeros((B, 2), dtype=np.float64)
    for k in range(N_CORES):
        out += res[k]["out"].astype(np.float64)
    out += c
    return out.astype(np.float32)


# revision 10
# speedup vs baseline: 1.2383x; 1.2383x over previous
"""Trainium2 Bass kernel for nn_LCNSpiking_58162447123130.

Math (verified to rel-err ~2e-3 against the oracle):

1. The "spiking" update carries zero state (syn = ALPHA*0 + cur, mem =
   BETA*0 + syn, reset = 0), so each LCN layer is a purely LINEAR map:
   h_out = h @ S_l + b_l with S_l[knn_l[j,k], j] = w_l[j,k].
2. Only the last timestep reaches the output, so
   out = x[:, -1, :] @ M + c with M = S0 S1 S2 S3 S4 Wfc ([14400, 2]),
   folded on the host in float64 from the tiny weight/index tables.

Device kernel (per core, 8-way split of the 14400 contraction axis):
stream a packed bf16 [1800, 34] tile (cols 0:32 = x_t slice, cols
32:34 = M slice) through one HWDGE DMA on the sync queue, 15
accumulating [K=120] x [32|2] bf16 matmuls on PE into a single fp32
PSUM tile, DVE copy PSUM->SBUF, sync-queue DMA of the [32, 2] partial
back to DRAM; the host sums the 8 partials (+ bias chain) in fp64.

bf16 operands halve the DMA payload and run PE at 1 cycle/row (fp32
needs the LOW/HIGH double pump); the fp32 PSUM accumulator keeps the
dot-product error ~2e-3, well inside the 2e-2 gate.

The four bass-preamble constant-MEMSETs (const-float32-0.0 etc.) are
dead code for this kernel - nothing reads the constant tiles - so they
are dropped from the entry block before compile.
"""

import numpy as np
import ml_dtypes

N_CORES = 8
B = 32                      # batch
D = 14400                   # layer-0 input dim
PER_CORE = D // N_CORES     # 1800 contraction elements per core
P = 120                     # SBUF partitions used (1800 = 120 * 15)
CHUNKS = PER_CORE // P      # 15 matmul accumulation steps
W = B + 2                   # packed row width: 32 x cols + 2 m cols
DIMS_IN = [14400, 7200, 3600, 1800, 900]

_compiled_nc = None


def _build_nc():
    import concourse.bass as bass
    import concourse.mybir as mybir

    dt = mybir.dt.bfloat16
    nc = bass.Bass()
    xm = nc.declare_dram_parameter("xm", [PER_CORE, W], dt, isOutput=False)
    out = nc.declare_dram_parameter("out", [B, 2], mybir.dt.float32, isOutput=True)

    with (
        nc.Block(no_gpsimd_drain=True) as block,
        nc.semaphore("sq_sem") as sq_sem,
        nc.semaphore("pq_sem") as pq_sem,
        nc.semaphore("pe_sem") as pe_sem,
        nc.semaphore("cp_sem") as cp_sem,
        nc.sbuf_tensor("t", [P, CHUNKS * W], dt) as t,
        nc.sbuf_tensor("ot", [B, 2], mybir.dt.float32) as ot,
        nc.psum_tensor("ps", [B, 2], mybir.dt.float32) as ps,
        nc.psum_tensor("ps_warm", [B, 2], mybir.dt.float32) as ps_warm,
    ):
        @block.sync
        def _(sync):
            # One DMA covers both operands: partition p holds contraction
            # rows [15p, 15p+15) as 15 chunks of 34 bf16 (1020 B contiguous
            # per partition on both sides).
            sync.dma_start(
                out=t[:, :].rearrange("p (c n) -> p c n", c=CHUNKS),
                in_=xm[:, :].rearrange("(p c) n -> p c n", p=P),
            ).then_inc(sq_sem, 16)

        @block.gpsimd
        def _(gpsimd):
            # Final store from gpsimd (SWDGE): its Block-exit drain is
            # skipped (no_gpsimd_drain), so the exit barrier isn't held up
            # by the ~370ns HWDGE drain the sync engine would pay after
            # dispatching this DMA.  The runtime's model-completion drain
            # covers the in-flight transfer, so no completion wait here.
            # The already-satisfied waits below keep the Pool sequencer
            # awake through the PE chain - cold it takes ~700ns to wake
            # for the store dispatch.
            for _ in range(8):
                gpsimd.wait_ge(sq_sem, 16)
            gpsimd.wait_ge(cp_sem, 1)
            gpsimd.dma_start(out=out[:, :], in_=ot[:, :]).then_inc(pq_sem, 16)

        @block.tensor
        def _(tensor):
            tensor.wait_ge(sq_sem, 16)
            mm = None
            for c in range(CHUNKS):
                # psum[32, 2] += t[:, c, 0:32].T @ t[:, c, 32:34]
                mm = nc.tensor.matmul(
                    ps[:, :],
                    t[:, c * W : c * W + B],
                    t[:, c * W + B : c * W + W],
                    start=(c == 0),
                    stop=(c == CHUNKS - 1),
                )
            mm.then_inc(pe_sem, 1)
            # Keep the PE sequencer at its warm p-state through the
            # copy/store tail: the NEFF epilogue's per-semaphore reset
            # chain on PE (51 instructions) dispatches measurably faster
            # when PE was recently busy.  Scratch matmuls into an unread
            # PSUM bank; sized to finish before the store dispatch does,
            # so PE is never the last engine into the exit barrier.
            for _ in range(30):
                nc.tensor.matmul(
                    ps_warm[:, :],
                    t[:, 0:B],
                    t[:, B:W],
                    start=True,
                    stop=True,
                )

        @block.vector
        def _(vector):
            vector.wait_ge(pe_sem, 1)
            nc.vector.tensor_copy(ot[:, :], ps[:, :]).then_inc(cp_sem, 1)

    # Drop the unused bass constant-tile memsets (dead code here).
    entry = nc.m.functions[0].blocks[0]
    entry.instructions = [
        i for i in entry.instructions if not isinstance(i, mybir.InstMemset)
    ]
    return nc


def _get_nc():
    global _compiled_nc
    if _compiled_nc is None:
        _compiled_nc = _build_nc()
    return _compiled_nc


def _fold(inputs):
    """Collapse the linear layer chain into M [14400, 2] and bias c [2]."""
    M = np.asarray(inputs["Wfc"]).astype(np.float64)
    c = np.asarray(inputs["bfc"]).astype(np.float64)
    for l in (4, 3, 2, 1, 0):
        knn = np.asarray(inputs[f"knn{l}"]).reshape(-1)
        w = np.asarray(inputs[f"w{l}"]).astype(np.float64)
        b = np.asarray(inputs[f"b{l}"]).astype(np.float64)
        c = (b @ M).ravel() + c
        Mnew = np.zeros((DIMS_IN[l], M.shape[1]), dtype=np.float64)
        np.add.at(Mnew, knn, (w[:, :, None] * M[:, None, :]).reshape(-1, M.shape[1]))
        M = Mnew
    return M.astype(np.float32), c


def kernel(**inputs) -> np.ndarray:
    from concourse.bass_utils import run_bass_kernel_spmd

    x = np.asarray(inputs["x"], dtype=np.float32)
    M, c = _fold(inputs)

    # Only the last timestep reaches the output; ship it transposed so the
    # contraction dim lands on SBUF partitions, packed next to the M slice.
    packed = np.empty((D, W), dtype=ml_dtypes.bfloat16)
    packed[:, :B] = x[:, -1, :].T.astype(ml_dtypes.bfloat16)
    packed[:, B:] = M.astype(ml_dtypes.bfloat16)

    nc = _get_nc()
    in_maps = [
        {"xm": packed[k * PER_CORE : (k + 1) * PER_CORE]}
        for k in range(N_CORES)
    ]
    res = run_bass_kernel_spmd(nc, in_maps, list(range(N_CORES))).results
    out = np.zeros((B, 2), dtype=np.float64)
    for k in range(N_CORES):
        out += res[k]["out"].astype(np.float64)
    out += c
    return out.astype(np.float32)


# revision 11
# speedup vs baseline: 1.2427x; 1.0035x over previous
"""Trainium2 Bass kernel for nn_LCNSpiking_58162447123130.

Math (verified to rel-err ~2e-3 against the oracle):

1. The "spiking" update carries zero state (syn = ALPHA*0 + cur, mem =
   BETA*0 + syn, reset = 0), so each LCN layer is a purely LINEAR map:
   h_out = h @ S_l + b_l with S_l[knn_l[j,k], j] = w_l[j,k].
2. Only the last timestep reaches the output, so
   out = x[:, -1, :] @ M + c with M = S0 S1 S2 S3 S4 Wfc ([14400, 2]),
   folded on the host in float64 from the tiny weight/index tables.

Device kernel (per core, 8-way split of the 14400 contraction axis):
stream a packed bf16 [1800, 34] tile (cols 0:32 = x_t slice, cols
32:34 = M slice) through one HWDGE DMA on the sync queue, 15
accumulating [K=120] x [32|2] bf16 matmuls on PE into a single fp32
PSUM tile, DVE copy PSUM->SBUF, sync-queue DMA of the [32, 2] partial
back to DRAM; the host sums the 8 partials (+ bias chain) in fp64.

bf16 operands halve the DMA payload and run PE at 1 cycle/row (fp32
needs the LOW/HIGH double pump); the fp32 PSUM accumulator keeps the
dot-product error ~2e-3, well inside the 2e-2 gate.

The four bass-preamble constant-MEMSETs (const-float32-0.0 etc.) are
dead code for this kernel - nothing reads the constant tiles - so they
are dropped from the entry block before compile.
"""

import numpy as np
import ml_dtypes

N_CORES = 8
B = 32                      # batch
D = 14400                   # layer-0 input dim
PER_CORE = D // N_CORES     # 1800 contraction elements per core
P = 120                     # SBUF partitions used (1800 = 120 * 15)
CHUNKS = PER_CORE // P      # 15 matmul accumulation steps
W = B + 2                   # packed row width: 32 x cols + 2 m cols
DIMS_IN = [14400, 7200, 3600, 1800, 900]

_compiled_nc = None


def _build_nc():
    import concourse.bass as bass
    import concourse.mybir as mybir

    dt = mybir.dt.bfloat16
    nc = bass.Bass()
    xm = nc.declare_dram_parameter("xm", [PER_CORE, W], dt, isOutput=False)
    out = nc.declare_dram_parameter("out", [B, 2], mybir.dt.float32, isOutput=True)

    with (
        nc.Block(no_gpsimd_drain=True) as block,
        nc.semaphore("sq_sem") as sq_sem,
        nc.semaphore("pq_sem") as pq_sem,
        nc.semaphore("pe_sem") as pe_sem,
        nc.semaphore("cp_sem") as cp_sem,
        nc.sbuf_tensor("t", [P, CHUNKS * W], dt) as t,
        nc.sbuf_tensor("ot", [B, 2], mybir.dt.float32) as ot,
        nc.psum_tensor("ps", [B, 2], mybir.dt.float32) as ps,
        nc.psum_tensor("ps_warm", [B, 2], mybir.dt.float32) as ps_warm,
    ):
        @block.sync
        def _(sync):
            # One DMA covers both operands: partition p holds contraction
            # rows [15p, 15p+15) as 15 chunks of 34 bf16 (1020 B contiguous
            # per partition on both sides).
            sync.dma_start(
                out=t[:, :].rearrange("p (c n) -> p c n", c=CHUNKS),
                in_=xm[:, :].rearrange("(p c) n -> p c n", p=P),
            ).then_inc(sq_sem, 16)

        @block.gpsimd
        def _(gpsimd):
            # Final store from gpsimd (SWDGE): its Block-exit drain is
            # skipped (no_gpsimd_drain), so the exit barrier isn't held up
            # by the ~370ns HWDGE drain the sync engine would pay after
            # dispatching this DMA.  The runtime's model-completion drain
            # covers the in-flight transfer, so no completion wait here.
            # The already-satisfied waits below keep the Pool sequencer
            # awake through the PE chain - cold it takes ~700ns to wake
            # for the store dispatch.
            for _ in range(8):
                gpsimd.wait_ge(sq_sem, 16)
            # Block on pe_sem first: Pool pays its ~600ns post-wait resume
            # stall starting at matmul-end, concurrently with the DVE copy,
            # so the cp_sem wait below is already satisfied (fast resume).
            gpsimd.wait_ge(pe_sem, 1)
            gpsimd.wait_ge(cp_sem, 1)
            gpsimd.dma_start(out=out[:, :], in_=ot[:, :]).then_inc(pq_sem, 16)

        @block.tensor
        def _(tensor):
            tensor.wait_ge(sq_sem, 16)
            mm = None
            for c in range(CHUNKS):
                # psum[32, 2] += t[:, c, 0:32].T @ t[:, c, 32:34]
                mm = nc.tensor.matmul(
                    ps[:, :],
                    t[:, c * W : c * W + B],
                    t[:, c * W + B : c * W + W],
                    start=(c == 0),
                    stop=(c == CHUNKS - 1),
                )
            mm.then_inc(pe_sem, 1)
            # Keep the PE sequencer at its warm p-state through the
            # copy/store tail: the NEFF epilogue's per-semaphore reset
            # chain on PE (51 instructions) dispatches measurably faster
            # when PE was recently busy.  Scratch matmuls into an unread
            # PSUM bank; sized to finish before the store dispatch does,
            # so PE is never the last engine into the exit barrier.
            for _ in range(30):
                nc.tensor.matmul(
                    ps_warm[:, :],
                    t[:, 0:B],
                    t[:, B:W],
                    start=True,
                    stop=True,
                )

        @block.vector
        def _(vector):
            vector.wait_ge(pe_sem, 1)
            nc.vector.tensor_copy(ot[:, :], ps[:, :]).then_inc(cp_sem, 1)

    # Drop the unused bass constant-tile memsets (dead code here).
    entry = nc.m.functions[0].blocks[0]
    entry.instructions = [
        i for i in entry.instructions if not isinstance(i, mybir.InstMemset)
    ]
    return nc


def _get_nc():
    global _compiled_nc
    if _compiled_nc is None:
        _compiled_nc = _build_nc()
    return _compiled_nc


def _fold(inputs):
    """Collapse the linear layer chain into M [14400, 2] and bias c [2]."""
    M = np.asarray(inputs["Wfc"]).astype(np.float64)
    c = np.asarray(inputs["bfc"]).astype(np.float64)
    for l in (4, 3, 2, 1, 0):
        knn = np.asarray(inputs[f"knn{l}"]).reshape(-1)
        w = np.asarray(inputs[f"w{l}"]).astype(np.float64)
        b = np.asarray(inputs[f"b{l}"]).astype(np.float64)
        c = (b @ M).ravel() + c
        Mnew = np.zeros((DIMS_IN[l], M.shape[1]), dtype=np.float64)
        np.add.at(Mnew, knn, (w[:, :, None] * M[:, None, :]).reshape(-1, M.shape[1]))
        M = Mnew
    return M.astype(np.float32), c


def kernel(**inputs) -> np.ndarray:
    from concourse.bass_utils import run_bass_kernel_spmd

    x = np.asarray(inputs["x"], dtype=np.float32)
    M, c = _fold(inputs)

    # Only the last timestep reaches the output; ship it transposed so the
    # contraction dim lands on SBUF partitions, packed next to the M slice.
    packed = np.empty((D, W), dtype=ml_dtypes.bfloat16)
    packed[:, :B] = x[:, -1, :].T.astype(ml_dtypes.bfloat16)
    packed[:, B:] = M.astype(ml_dtypes.bfloat16)

    nc = _get_nc()
    in_maps = [
        {"xm": packed[k * PER_CORE : (k + 1) * PER_CORE]}
        for k in range(N_CORES)
    ]
    res = run_bass_kernel_spmd(nc, in_maps, list(range(N_CORES))).results
    out = np.zeros((B, 2), dtype=np.float64)
    for k in range(N_CORES):
        out += res[k]["out"].astype(np.float64)
    out += c
    return out.astype(np.float32)
